# revision 56
# baseline (speedup 1.0000x reference)
"""FlowNetC correlation kernel for Trainium2 (8 NeuronCores, data-parallel over batch).

Problem: out[b, d, y, x] = (1/256) * sum_c in1[b,c,y,x] * in2pad[b,c,y+dy,x+dx]
  with in2 zero-padded by 20 on each spatial side, d = 21*dyi + dxi,
  dy = 2*dyi - 20, dx = 2*dxi - 20 (21x21 = 441 displacements, stride 2).
Shapes: in1/in2 [16, 256, 48, 64] f32 -> out [16, 441, 48, 64] f32.

Strategy per core (2 images):
  - Parity-split rows/cols (displacements are stride-2, so x couples only to
    same-parity padded cols).  24 blocks per image of M=128 = 8 same-parity
    rows x 16 same-parity cols.  PE computes Gram blocks
    G[m=(ys,x_e), n=(rs,u)] = sum_c A[c,y,x] * B[c,rp,xp] as bf16 matmuls
    (1 cycle/row vs 4 for fp32; inputs are cast to bf16 on-chip, which also
    halves staging SBUF and scratch DMA traffic).  Weights are packed into a
    per-block-contiguous apk tile (PE weight APs allow only one free dim).
  - DVE/Act copy PSUM into a per-image bf16 staging tile sg with the 24
    blocks INTERLEAVED innermost ([(rs,u)][blk]), folding the 1/256
    normalization into the copy; never-written pad-u stripes are pre-zeroed
    once.  Image 1's prep (casts+packs) runs on Pool so it never blocks
    image 0's scatter stream; image 1's scatters run DVE-only so image 0's
    shuffles (Act+Pool) aren't starved.
  - Row-diagonal applied at WRITE time: per (image, ys) one DMA writes only
    the 21-row rs window [ys, ys+21) of partitions m=(ys,:) to DRAM scratch
    (75% of sg, vs 100% for a full dump).  Scratch layout [ys][x_e][dyi][u*blk].
  - Column-diagonal applied at GATHER time: u = x_e + dxi is an affine offset
    on the DRAM side; with blk innermost (dxi, blk) merge into 1008B runs.
    One gather DMA per (image, ys) into s3 tiles with partitions = (slice,
    dyi) - four (b, ys) slices packed per 84-partition group.
  - Act/Pool (img 0) / DVE/Act/Pool (img 1) free-dim shuffle to x-contiguous
    order into per-yt s4 tiles, then final DMAs to the d-major output with
    512B runs.  Loads for image 1 are emitted right after image 0's prep so
    their transfers fill DMA idle time during image 0's matmul phase.

Cost-model timeline per core: 128.6us (from 219.0us baseline).  DMA busy
~105.9us is the binding resource (loads 35 + scratch write 25.8 + gather 15
+ output 30); PE ~39us, engines ~30-45us each, all hidden under DMA.
"""

import numpy as np

_CACHE = {}

# ---- geometry (hardcoded for [16, 256, 48, 64]) ----
N_CORES = 8
B2 = 2            # images per core
CH = 2            # channel chunks
CP = 128          # channels per chunk (partition dim)
H, W = 48, 64
PY, YT, YS = 2, 3, 8     # y parity, y tiles, rows per block
PX, XT, XE = 2, 2, 16    # x parity, x tiles, cols per block
NDI = 21                 # dyi / dxi count
RS, U = 28, 36           # rhs window rows / cols (parity space)
NBLK = PY * YT * PX * XT          # 24 blocks per image
RP = 88                           # padded in2 rows
NN = RS * U                       # 1008 sg free rows (rs,u)
SGF = NN * NBLK + NBLK            # sg pitch per partition (24216); one-blk
                                  # pad keeps the AP optimizer from merging
                                  # the ys partition dim with the rs free dim
                                  # into a non-partition-aligned stride
WRUN = NDI * U * NBLK             # write run per partition = 18144
SCRS = XE * WRUN                  # scratch elems per (b, ys) = 169344
S3B = XE * NDI * NBLK             # s3 free = 8064
S4F = NDI * PY * W                # per-yt s4 free = 2688


def _build():
    import concourse.bacc as bacc
    import concourse.bass as bass
    import concourse.mybir as mybir
    import concourse.tile as tile

    f32 = mybir.dt.float32
    f32r = mybir.dt.float32r
    bf16 = mybir.dt.bfloat16
    COPY = mybir.ActivationFunctionType.Copy
    nc = bacc.Bacc("TRN2", target_bir_lowering=False, debug=False,
                   enable_asserts=False, num_devices=N_CORES)

    in1 = nc.dram_tensor("in1", [B2, CH * CP, H, W], f32, kind="ExternalInput")
    in2 = nc.dram_tensor("in2", [B2, CH * CP, H, W], f32, kind="ExternalInput")
    out = nc.dram_tensor("out", [B2, NDI * NDI, H, W], f32, kind="ExternalOutput")

    with tile.TileContext(nc) as tc:
        with (
            tc.tile_pool(name="io", bufs=1) as io_pool,
            tc.tile_pool(name="s3p", bufs=1) as s3_pool,
            tc.tile_pool(name="apkp", bufs=1) as apk_pool,
            tc.tile_pool(name="s4p", bufs=1) as s4_pool,
            tc.tile_pool(name="psum", bufs=4, space="PSUM") as psum_pool,
        ):
            global _DEBUG_POOLS
            _DEBUG_POOLS = [io_pool, s3_pool, apk_pool, s4_pool]
            stg1 = io_pool.tile([CP, CH * H * W], f32)     # 24.6KB/part
            stg2 = io_pool.tile([CP, CH * H * W], f32)     # 24.6KB/part
            bsb = io_pool.tile([CP, CH, RP, W], bf16)      # 22.5KB/part
            sg = io_pool.tile([CP, SGF], bf16)             # 47.3KB/part

            # one-time zeroing: in2 pad rows; sg pad-u stripes (never written
            # by scatters: xt=0 blocks own u in [0,10), xt=1 u in [26,36))
            nc.gpsimd.memset(bsb[:, :, 0:20, :], 0.0)
            nc.gpsimd.memset(bsb[:, :, 68:88, :], 0.0)
            nc.gpsimd.memset(
                bass.AP(sg.tensor, 0,
                        [[SGF, CP], [2, NBLK // 2], [U * NBLK, RS], [NBLK, 10]]),
                0.0)
            nc.gpsimd.memset(
                bass.AP(sg.tensor, 26 * NBLK + 1,
                        [[SGF, CP], [2, NBLK // 2], [U * NBLK, RS], [NBLK, 10]]),
                0.0)

            dmae = [nc.sync, nc.scalar, nc.sync]

            def emit_loads(b):
                """per-ch-chunk loads on SP, in2 first (bsb casts are the
                longer downstream chain)."""
                for stg, src_t in ((stg2, in2), (stg1, in1)):
                    for ch in range(CH):
                        nc.sync.dma_start(
                            bass.AP(stg.tensor, ch * H * W,
                                    [[CH * H * W, CP], [1, H * W]]),
                            bass.AP(src_t, (b * CH + ch) * CP * H * W,
                                    [[H * W, CP], [1, H * W]]))

            def emit_prep(b, apk):
                """casts + weight packs for image b.  b=0 runs on DVE+Act
                (start of kernel, both idle); b=1 runs on Pool only so it
                never blocks image 0's scatter/shuffle stream.  Packs order
                the PE output partitions as m = x_e*8 + ys (x_e-major) so
                scratch writes can split per-x_e with an exact u-window."""
                engs = ([nc.vector, nc.scalar] if b == 0
                        else [nc.gpsimd, nc.vector, nc.scalar])
                k = 0
                for ch in range(CH):
                    for py in range(PY):
                        for px in range(PX):
                            for xt in range(XT):
                                psrc = bass.AP(
                                    stg1.tensor,
                                    ch * H * W + py * W + px + 32 * xt,
                                    [[CH * H * W, CP], [16 * W, YT],
                                     [2 * W, YS], [2, XE]])
                                pdst = bass.AP(
                                    apk.tensor,
                                    (((ch * PY + py) * PX + px) * XT + xt)
                                    * YT * 128,
                                    [[CH * PY * PX * XT * YT * 128, CP],
                                     [128, YT], [1, YS], [8, XE]])
                                e = engs[k % len(engs)]; k += 1
                                if e is nc.scalar:
                                    nc.scalar.activation(pdst, psrc, COPY)
                                else:
                                    e.tensor_copy(pdst, psrc)
                # bsb = in2 / 256: folding the normalization into the cast
                # (exponent shift, lossless in bf16) makes the PSUM->sg
                # scatters pure copies.
                for ch in range(CH):
                    c2dst = bsb[:, ch, 20:20 + H, :]
                    c2src = bass.AP(stg2.tensor, ch * H * W,
                                    [[CH * H * W, CP], [1, H * W]])
                    if ch == 0:
                        nc.vector.tensor_scalar_mul(c2dst, c2src, 1.0 / 256.0)
                    else:
                        nc.scalar.activation(c2dst, c2src, COPY,
                                             scale=1.0 / 256.0)

            def emit_blocks(b, apk):
                """matmuls + PSUM->sg scatters for image b (PE; DVE+Act).
                b=1 scatters lean 2:1 on DVE (Act is busier with image-0
                shuffles in that window)."""
                for py in range(PY):
                    for yt in range(YT):
                        y0 = yt * 16 + py
                        for px in range(PX):
                            for xt in range(XT):
                                x0 = xt * 32 + px
                                blk = ((py * YT + yt) * PX + px) * XT + xt
                                u_lo = 10 if xt == 0 else 0
                                xp0 = x0 + 2 * u_lo - 20
                                ps = psum_pool.tile([128, 1024], f32)
                                for ch in range(CH):
                                    lt = apk[:, ch, py, px, xt, yt, :]
                                    for h in range(2):
                                        rhs = bsb[:, ch,
                                                  y0 + 28 * h:y0 + 28 * h + 27:2,
                                                  xp0:xp0 + 51:2]
                                        nc.tensor.matmul(
                                            ps[:, 512 * h:512 * h + 364],
                                            lt, rhs,
                                            start=(ch == 0), stop=(ch == CH - 1))
                                csrc = bass.AP(ps.tensor, 0,
                                               [[1024, 128], [512, 2],
                                                [26, 14], [1, 26]])
                                cdst = bass.AP(sg.tensor, u_lo * NBLK + blk,
                                               [[SGF, 128], [14 * U * NBLK, 2],
                                                [U * NBLK, 14], [NBLK, 26]])
                                if (blk % 2 == 0) if b == 0 else (blk % 3 < 2):
                                    nc.vector.tensor_copy(cdst, csrc)
                                else:
                                    nc.scalar.activation(cdst, csrc, COPY)

            # sg -> s3 direct SBUF->SBUF DMAs, no DRAM bounce: the u-window
            # [x_e, x_e+21) is an exact free-dim window (504-elem runs); the
            # rs-diagonal is folded into the DST PARTITION index q = s*28+rs
            # (exact partition steps, so the BIR verifier accepts).  The
            # valid band lands at q = s*29 + 4g + dyi; out-of-window rs rows
            # land on junk partitions the output DMAs never read.
            def emit_diag_g(b, g, s3):
                """sg -> s3 group-g half: 16 SBUF->SBUF DMAs, one per x_e."""
                # issue seats: the two HWDGE queues (SP, Act) + a Pool SWDGE
                # share (Pool bypasses the shared HWDGE device, which would
                # otherwise serialize these 314ns transfers at 630ns each).
                seats = [nc.sync, nc.scalar, nc.gpsimd] * 5 + [nc.gpsimd]
                for xe in range(XE):
                    src = bass.AP(sg.tensor,
                                  (xe * 8 + 4 * g) * SGF + xe * NBLK,
                                  [[SGF, 4],          # ys partitions
                                   [U * NBLK, RS],    # rs (28)
                                   [1, NDI * NBLK]])  # u-window x blk
                    dst = bass.AP(s3.tensor, g * S3B + xe * NDI * NBLK,
                                  [[2 * S3B, 112],    # q = s*28 + rs
                                   [1, NDI * NBLK]])
                    seats[(g * XE + xe) % len(seats)].dma_start(dst, src)

            def emit_diag(b):
                s3 = s3_pool.tile([128, 2 * S3B], bf16, tag="s3")
                emit_diag_g(b, 0, s3)
                emit_diag_g(b, 1, s3)
                return s3

            def emit_shuffles(b, s3):
                """s3 -> s4 x-interleave + bf16->f32 cast over all 112
                diag-written partitions (junk bands shuffle along harmlessly),
                then final DMAs reading only the valid q = s*29 + 4g + dyi
                partition bands."""
                cpe = 0
                for g in range(2):
                    for yt in range(YT):
                        s4 = s4_pool.tile([128, S4F], f32,
                                          tag=f"s4_{(g * YT + yt) % 3}")
                        for py in range(PY):
                            for px in range(PX):
                                blk0 = ((py * YT + yt) * PX + px) * XT
                                ssrc = bass.AP(
                                    s3.tensor, g * S3B + blk0,
                                    [[2 * S3B, 112],
                                     [NDI * NBLK, XE],
                                     [1, XT],
                                     [NBLK, NDI]])
                                sdst = bass.AP(
                                    s4.tensor, py * W + px,
                                    [[S4F, 112],
                                     [2, XE],
                                     [32, XT],
                                     [PY * W, NDI]])
                                if b == 0:
                                    eng = 1 + cpe % 2   # Act/Pool
                                else:
                                    eng = cpe % 3       # DVE/Act/Pool
                                cpe += 1
                                if eng == 0:
                                    nc.vector.tensor_copy(sdst, ssrc)
                                elif eng == 1:
                                    nc.scalar.activation(sdst, ssrc, COPY)
                                else:
                                    nc.gpsimd.tensor_copy(sdst, ssrc)
                        for s in range(4):
                            ys = g * 4 + s
                            y = yt * 16 + 2 * ys
                            fsrc = bass.AP(s4.tensor,
                                           (s * 29 + 4 * g) * S4F,
                                           [[S4F, NDI], [PY * W, NDI],
                                            [1, PY * W]])
                            # valid band q = s*29 + 4g + dyi
                            fdst = bass.AP(out, b * 441 * H * W + y * W,
                                           [[NDI * H * W, NDI],
                                            [H * W, NDI],
                                            [1, PY * W]])
                            dmae[(s + yt * 4 + g * 12) % 3].dma_start(fdst, fsrc)

            apks = []
            for b in range(B2):
                apk_b = apk_pool.tile([CP, CH, PY, PX, XT, YT, 128], bf16,
                                      tag=f"apk{b}")
                apks.append(apk_b)

            emit_loads(0)
            emit_prep(0, apks[0])       # DVE+Act; packs free stg1 first
            emit_loads(1)               # WAR on stg1/stg2: fires after prep_b0
            emit_blocks(0, apks[0])
            emit_prep(1, apks[1])       # Pool only; runs during image 0
            g0 = emit_diag(0)
            emit_blocks(1, apks[1])     # DVE/Act scatters before b0 shuffles
            emit_shuffles(0, g0)
            g1 = emit_diag(1)
            emit_shuffles(1, g1)

    import os
    if os.environ.get("KERNEL_DEBUG_POOLS"):
        for p in _DEBUG_POOLS:
            p.print_usage()
    nc.compile()
    return nc


def _get_nc():
    if "nc" not in _CACHE:
        _CACHE["nc"] = _build()
    return _CACHE["nc"]


def kernel(input1, input2):
    from concourse.bass_utils import run_bass_kernel_spmd

    input1 = np.ascontiguousarray(np.asarray(input1), dtype=np.float32)
    input2 = np.ascontiguousarray(np.asarray(input2), dtype=np.float32)
    nc = _get_nc()
    in_maps = [
        {"in1": input1[i * B2:(i + 1) * B2], "in2": input2[i * B2:(i + 1) * B2]}
        for i in range(N_CORES)
    ]
    res = run_bass_kernel_spmd(nc, in_maps, list(range(N_CORES)))
    return np.concatenate([res.results[i]["out"] for i in range(N_CORES)], axis=0)



# revision 61
# speedup vs baseline: 1.0341x; 1.0341x over previous
"""FlowNetC correlation kernel for Trainium2 (8 NeuronCores, data-parallel over batch).

Problem: out[b, d, y, x] = (1/256) * sum_c in1[b,c,y,x] * in2pad[b,c,y+dy,x+dx]
  with in2 zero-padded by 20 on each spatial side, d = 21*dyi + dxi,
  dy = 2*dyi - 20, dx = 2*dxi - 20 (21x21 = 441 displacements, stride 2).
Shapes: in1/in2 [16, 256, 48, 64] f32 -> out [16, 441, 48, 64] f32.

Strategy per core (2 images):
  - Parity-split rows/cols (displacements are stride-2, so x couples only to
    same-parity padded cols).  24 blocks per image of M=128 = 8 same-parity
    rows x 16 same-parity cols.  PE computes Gram blocks
    G[m=(ys,x_e), n=(rs,u)] = sum_c A[c,y,x] * B[c,rp,xp] as bf16 matmuls
    (1 cycle/row vs 4 for fp32; inputs are cast to bf16 on-chip, which also
    halves staging SBUF and scratch DMA traffic).  Weights are packed into a
    per-block-contiguous apk tile (PE weight APs allow only one free dim).
  - DVE/Act copy PSUM into a per-image bf16 staging tile sg with the 24
    blocks INTERLEAVED innermost ([(rs,u)][blk]), folding the 1/256
    normalization into the copy; never-written pad-u stripes are pre-zeroed
    once.  Image 1's prep (casts+packs) runs on Pool so it never blocks
    image 0's scatter stream; image 1's scatters run DVE-only so image 0's
    shuffles (Act+Pool) aren't starved.
  - Row-diagonal applied at WRITE time: per (image, ys) one DMA writes only
    the 21-row rs window [ys, ys+21) of partitions m=(ys,:) to DRAM scratch
    (75% of sg, vs 100% for a full dump).  Scratch layout [ys][x_e][dyi][u*blk].
  - Column-diagonal applied at GATHER time: u = x_e + dxi is an affine offset
    on the DRAM side; with blk innermost (dxi, blk) merge into 1008B runs.
    One gather DMA per (image, ys) into s3 tiles with partitions = (slice,
    dyi) - four (b, ys) slices packed per 84-partition group.
  - Act/Pool (img 0) / DVE/Act/Pool (img 1) free-dim shuffle to x-contiguous
    order into per-yt s4 tiles, then final DMAs to the d-major output with
    512B runs.  Loads for image 1 are emitted right after image 0's prep so
    their transfers fill DMA idle time during image 0's matmul phase.

Cost-model timeline per core: 128.6us (from 219.0us baseline).  DMA busy
~105.9us is the binding resource (loads 35 + scratch write 25.8 + gather 15
+ output 30); PE ~39us, engines ~30-45us each, all hidden under DMA.
"""

import numpy as np

_CACHE = {}

# ---- geometry (hardcoded for [16, 256, 48, 64]) ----
N_CORES = 8
B2 = 2            # images per core
CH = 2            # channel chunks
CP = 128          # channels per chunk (partition dim)
H, W = 48, 64
PY, YT, YS = 2, 3, 8     # y parity, y tiles, rows per block
PX, XT, XE = 2, 2, 16    # x parity, x tiles, cols per block
NDI = 21                 # dyi / dxi count
RS, U = 28, 36           # rhs window rows / cols (parity space)
NBLK = PY * YT * PX * XT          # 24 blocks per image
RP = 88                           # padded in2 rows
NN = RS * U                       # 1008 sg free rows (rs,u)
SGF = NN * NBLK + NBLK            # sg pitch per partition (24216); one-blk
                                  # pad keeps the AP optimizer from merging
                                  # the ys partition dim with the rs free dim
                                  # into a non-partition-aligned stride
WRUN = NDI * U * NBLK             # write run per partition = 18144
SCRS = XE * WRUN                  # scratch elems per (b, ys) = 169344
S3B = XE * NDI * NBLK             # s3 free = 8064
S4F = NDI * PY * W                # per-yt s4 free = 2688


def _build():
    import concourse.bacc as bacc
    import concourse.bass as bass
    import concourse.mybir as mybir
    import concourse.tile as tile

    f32 = mybir.dt.float32
    f32r = mybir.dt.float32r
    bf16 = mybir.dt.bfloat16
    COPY = mybir.ActivationFunctionType.Copy
    nc = bacc.Bacc("TRN2", target_bir_lowering=False, debug=False,
                   enable_asserts=False, num_devices=N_CORES)

    in1 = nc.dram_tensor("in1", [B2, CH * CP, H, W], f32, kind="ExternalInput")
    in2 = nc.dram_tensor("in2", [B2, CH * CP, H, W], f32, kind="ExternalInput")
    out = nc.dram_tensor("out", [B2, NDI * NDI, H, W], f32, kind="ExternalOutput")

    with tile.TileContext(nc) as tc:
        with (
            tc.tile_pool(name="io", bufs=1) as io_pool,
            tc.tile_pool(name="s3p", bufs=1) as s3_pool,
            tc.tile_pool(name="apkp", bufs=1) as apk_pool,
            tc.tile_pool(name="s4p", bufs=1) as s4_pool,
            tc.tile_pool(name="psum", bufs=4, space="PSUM") as psum_pool,
        ):
            global _DEBUG_POOLS
            _DEBUG_POOLS = [io_pool, s3_pool, apk_pool, s4_pool]
            stg1 = io_pool.tile([CP, CH * H * W], f32)     # 24.6KB/part
            stg2 = io_pool.tile([CP, CH * H * W], f32)     # 24.6KB/part
            bsb = io_pool.tile([CP, CH, RP, W], bf16)      # 22.5KB/part
            sg = io_pool.tile([CP, SGF], bf16)             # 47.3KB/part

            # one-time zeroing: in2 pad rows; sg pad-u stripes (never written
            # by scatters: xt=0 blocks own u in [0,10), xt=1 u in [26,36))
            nc.gpsimd.memset(bsb[:, :, 0:20, :], 0.0)
            nc.gpsimd.memset(bsb[:, :, 68:88, :], 0.0)
            nc.gpsimd.memset(
                bass.AP(sg.tensor, 0,
                        [[SGF, CP], [2, NBLK // 2], [U * NBLK, RS], [NBLK, 10]]),
                0.0)
            nc.gpsimd.memset(
                bass.AP(sg.tensor, 26 * NBLK + 1,
                        [[SGF, CP], [2, NBLK // 2], [U * NBLK, RS], [NBLK, 10]]),
                0.0)

            dmae = [nc.sync]

            def emit_loads(b):
                """per-ch-chunk loads on SP, in2 first (bsb casts are the
                longer downstream chain)."""
                for stg, src_t in ((stg2, in2), (stg1, in1)):
                    for ch in range(CH):
                        nc.sync.dma_start(
                            bass.AP(stg.tensor, ch * H * W,
                                    [[CH * H * W, CP], [1, H * W]]),
                            bass.AP(src_t, (b * CH + ch) * CP * H * W,
                                    [[H * W, CP], [1, H * W]]))

            def emit_prep(b, apk):
                """casts + weight packs for image b.  b=0 runs on DVE+Act
                (start of kernel, both idle); b=1 runs on Pool only so it
                never blocks image 0's scatter/shuffle stream.  Packs order
                the PE output partitions as m = x_e*8 + ys (x_e-major) so
                scratch writes can split per-x_e with an exact u-window."""
                engs = ([nc.vector, nc.scalar] if b == 0
                        else [nc.gpsimd, nc.vector, nc.scalar])
                k = 0
                for ch in range(CH):
                    for py in range(PY):
                        for px in range(PX):
                            for xt in range(XT):
                                psrc = bass.AP(
                                    stg1.tensor,
                                    ch * H * W + py * W + px + 32 * xt,
                                    [[CH * H * W, CP], [16 * W, YT],
                                     [2 * W, YS], [2, XE]])
                                pdst = bass.AP(
                                    apk.tensor,
                                    (((ch * PY + py) * PX + px) * XT + xt)
                                    * YT * 128,
                                    [[CH * PY * PX * XT * YT * 128, CP],
                                     [128, YT], [1, YS], [8, XE]])
                                e = engs[k % len(engs)]; k += 1
                                if e is nc.scalar:
                                    nc.scalar.activation(pdst, psrc, COPY)
                                else:
                                    e.tensor_copy(pdst, psrc)
                # bsb = in2 / 256: folding the normalization into the cast
                # (exponent shift, lossless in bf16) makes the PSUM->sg
                # scatters pure copies.
                for ch in range(CH):
                    c2dst = bsb[:, ch, 20:20 + H, :]
                    c2src = bass.AP(stg2.tensor, ch * H * W,
                                    [[CH * H * W, CP], [1, H * W]])
                    if ch == 0:
                        nc.vector.tensor_scalar_mul(c2dst, c2src, 1.0 / 256.0)
                    else:
                        nc.scalar.activation(c2dst, c2src, COPY,
                                             scale=1.0 / 256.0)

            def emit_blocks(b, apk):
                """matmuls + PSUM->sg scatters for image b (PE; DVE+Act).
                b=1 scatters lean 2:1 on DVE (Act is busier with image-0
                shuffles in that window)."""
                for py in range(PY):
                    for yt in range(YT):
                        y0 = yt * 16 + py
                        for px in range(PX):
                            for xt in range(XT):
                                x0 = xt * 32 + px
                                blk = ((py * YT + yt) * PX + px) * XT + xt
                                u_lo = 10 if xt == 0 else 0
                                xp0 = x0 + 2 * u_lo - 20
                                ps = psum_pool.tile([128, 1024], f32)
                                for ch in range(CH):
                                    lt = apk[:, ch, py, px, xt, yt, :]
                                    for h in range(2):
                                        rhs = bsb[:, ch,
                                                  y0 + 28 * h:y0 + 28 * h + 27:2,
                                                  xp0:xp0 + 51:2]
                                        nc.tensor.matmul(
                                            ps[:, 512 * h:512 * h + 364],
                                            lt, rhs,
                                            start=(ch == 0), stop=(ch == CH - 1))
                                csrc = bass.AP(ps.tensor, 0,
                                               [[1024, 128], [512, 2],
                                                [26, 14], [1, 26]])
                                cdst = bass.AP(sg.tensor, u_lo * NBLK + blk,
                                               [[SGF, 128], [14 * U * NBLK, 2],
                                                [U * NBLK, 14], [NBLK, 26]])
                                if blk % 2 == 0:
                                    nc.vector.tensor_copy(cdst, csrc)
                                else:
                                    nc.scalar.activation(cdst, csrc, COPY)

            # sg -> s3 direct SBUF->SBUF DMAs, no DRAM bounce: the u-window
            # [x_e, x_e+21) is an exact free-dim window (504-elem runs); the
            # rs-diagonal is folded into the DST PARTITION index q = s*28+rs
            # (exact partition steps, so the BIR verifier accepts).  The
            # valid band lands at q = s*29 + 4g + dyi; out-of-window rs rows
            # land on junk partitions the output DMAs never read.
            def emit_diag_g(b, g, s3):
                """sg -> s3 group-g half: 16 SBUF->SBUF DMAs, one per x_e."""
                # issue seats: the two HWDGE queues (SP, Act) + a Pool SWDGE
                # share (Pool bypasses the shared HWDGE device, which would
                # otherwise serialize these 314ns transfers at 630ns each).
                seats = [nc.sync, nc.scalar, nc.gpsimd] * 5 + [nc.gpsimd]
                for xe in range(XE):
                    src = bass.AP(sg.tensor,
                                  (xe * 8 + 4 * g) * SGF + xe * NBLK,
                                  [[SGF, 4],          # ys partitions
                                   [U * NBLK, RS],    # rs (28)
                                   [1, NDI * NBLK]])  # u-window x blk
                    dst = bass.AP(s3.tensor, g * S3B + xe * NDI * NBLK,
                                  [[2 * S3B, 112],    # q = s*28 + rs
                                   [1, NDI * NBLK]])
                    seats[(g * XE + xe) % len(seats)].dma_start(dst, src)

            def emit_diag(b):
                s3 = s3_pool.tile([128, 2 * S3B], bf16, tag="s3")
                emit_diag_g(b, 0, s3)
                emit_diag_g(b, 1, s3)
                return s3

            def emit_shuffles(b, s3):
                """s3 -> s4 x-interleave + bf16->f32 cast over all 112
                diag-written partitions (junk bands shuffle along harmlessly),
                then final DMAs reading only the valid q = s*29 + 4g + dyi
                partition bands."""
                cpe = 0
                for g in range(2):
                    for yt in range(YT):
                        s4 = s4_pool.tile([128, S4F], f32,
                                          tag=f"s4_{(g * YT + yt) % 3}")
                        for py in range(PY):
                            for px in range(PX):
                                blk0 = ((py * YT + yt) * PX + px) * XT
                                ssrc = bass.AP(
                                    s3.tensor, g * S3B + blk0,
                                    [[2 * S3B, 112],
                                     [NDI * NBLK, XE],
                                     [1, XT],
                                     [NBLK, NDI]])
                                sdst = bass.AP(
                                    s4.tensor, py * W + px,
                                    [[S4F, 112],
                                     [2, XE],
                                     [32, XT],
                                     [PY * W, NDI]])
                                eng = cpe % 3           # DVE/Act/Pool
                                cpe += 1
                                if eng == 0:
                                    nc.vector.tensor_copy(sdst, ssrc)
                                elif eng == 1:
                                    nc.scalar.activation(sdst, ssrc, COPY)
                                else:
                                    nc.gpsimd.tensor_copy(sdst, ssrc)
                        for s in range(4):
                            ys = g * 4 + s
                            y = yt * 16 + 2 * ys
                            fsrc = bass.AP(s4.tensor,
                                           (s * 29 + 4 * g) * S4F,
                                           [[S4F, NDI], [PY * W, NDI],
                                            [1, PY * W]])
                            # valid band q = s*29 + 4g + dyi
                            fdst = bass.AP(out, b * 441 * H * W + y * W,
                                           [[NDI * H * W, NDI],
                                            [H * W, NDI],
                                            [1, PY * W]])
                            dmae[0].dma_start(fdst, fsrc)

            apks = []
            for b in range(B2):
                apk_b = apk_pool.tile([CP, CH, PY, PX, XT, YT, 128], bf16,
                                      tag=f"apk{b}")
                apks.append(apk_b)

            emit_loads(0)
            emit_prep(0, apks[0])       # DVE+Act; packs free stg1 first
            emit_loads(1)               # WAR on stg1/stg2: fires after prep_b0
            emit_blocks(0, apks[0])
            emit_prep(1, apks[1])       # Pool only; runs during image 0
            g0 = emit_diag(0)
            emit_blocks(1, apks[1])     # DVE/Act scatters before b0 shuffles
            emit_shuffles(0, g0)
            g1 = emit_diag(1)
            emit_shuffles(1, g1)

    import os
    if os.environ.get("KERNEL_DEBUG_POOLS"):
        for p in _DEBUG_POOLS:
            p.print_usage()
    nc.compile()
    return nc


def _get_nc():
    if "nc" not in _CACHE:
        _CACHE["nc"] = _build()
    return _CACHE["nc"]


def kernel(input1, input2):
    from concourse.bass_utils import run_bass_kernel_spmd

    input1 = np.ascontiguousarray(np.asarray(input1), dtype=np.float32)
    input2 = np.ascontiguousarray(np.asarray(input2), dtype=np.float32)
    nc = _get_nc()
    in_maps = [
        {"in1": input1[i * B2:(i + 1) * B2], "in2": input2[i * B2:(i + 1) * B2]}
        for i in range(N_CORES)
    ]
    res = run_bass_kernel_spmd(nc, in_maps, list(range(N_CORES)))
    return np.concatenate([res.results[i]["out"] for i in range(N_CORES)], axis=0)

